# revision 1
# baseline (speedup 1.0000x reference)
"""Trainium2 Bass kernel for LongcatFlash MoE experts (expert-parallel, 8 cores).

Problem: T=4096 tokens, H=1024, I=512, 32 routed + 8 zero (identity) experts,
top-4 routing, per-expert capacity 768.

Strategy (sharding_hint = expert parallelism):
  - Host: compute routing (stable sort by expert, capacity clip), permute
    tokens to their expert's core (the "all-to-all"), build per-core packed
    activation buffers with tokens on the GEMM free dimension.
  - Device (8 cores, SPMD): each core owns 4 routed experts; per expert run
    the gated MLP as tiled matmuls:
        gu[o, c]  = sum_h guT[h, o] * xT[h, c]      (o = 2I rows, c = tokens)
        mid[i, c] = silu(gate[i, c]) * up[i, c]
        y[h, c]   = sum_i dnT[i, h] * mid[i, c]
    Tokens live on the free dim (N <= 512 per matmul), weights are the
    stationary operand.
  - Host: gather per-assignment outputs, scale by router weight, scatter-add
    back per token, add the zero-expert weighted-identity term.
"""

import math
import os

import numpy as np

N_CORES = 8
R = 32  # routed experts
E_PER_CORE = R // N_CORES  # 4
CAPACITY = 768
H = 1024
I_DIM = 512
HT = H // 128  # 8 h-tiles
OT = 2 * I_DIM // 128  # 8 o-tiles of gate_up
IT = I_DIM // 128  # 4 i-tiles

# precision mode: "bf16" (fast, rel err ~4e-4) or "f32r" (fp32 storage,
# FP22 matmul, rel err ~3e-5 but ~1.4x slower: 2 PE cycles/row + 2x DMA)
PREC = os.environ.get("MOE_PREC", "bf16")

LAST_RUN = {}  # filled with exec_time_ns etc. for test harness use


def _route(idx, wts, n_tok):
    """Replicates the reference's capacity-buffer routing exactly.

    Returns per-assignment (expert, token, weight, slot, flat_index) for kept
    routed assignments, sorted by expert (stable), plus zero-expert weights.
    """
    K = idx.shape[1]
    A = n_tok * K
    flat_e = idx.reshape(-1).astype(np.int64)
    flat_t = np.repeat(np.arange(n_tok, dtype=np.int64), K)
    flat_w = wts.reshape(-1)
    order = np.argsort(flat_e, kind="stable")
    se = flat_e[order]
    st = flat_t[order]
    sw = flat_w[order]
    counts = np.bincount(flat_e, minlength=R + 8)
    starts = np.cumsum(counts) - counts
    pos = np.arange(A, dtype=np.int64) - starts[se]
    valid = (se < R) & (pos < CAPACITY)
    zero_w = np.where(idx >= R, wts, 0.0).sum(axis=1)
    return (
        se[valid],
        st[valid],
        sw[valid],
        pos[valid],
        order[valid],
        zero_w,
    )


def _chunks(S):
    n = (S + 511) // 512
    base = S // n
    rem = S - base * n
    out = []
    c0 = 0
    for i in range(n):
        cn = base + (1 if i < rem else 0)
        out.append((c0, cn))
        c0 += cn
    return out


_BUILD_CACHE = {}


def _build_bass(S, prec):
    import concourse.bacc as bacc
    import concourse.bass as bass
    import concourse.mybir as mybir
    from concourse import tile

    key = (S, prec)
    if key in _BUILD_CACHE:
        return _BUILD_CACHE[key]

    FT = mybir.dt.float32
    if prec == "bf16":
        dram_dt = mybir.dt.bfloat16
        sb_dt = mybir.dt.bfloat16
        mid_dt = mybir.dt.bfloat16
        out_dt = mybir.dt.bfloat16
    else:
        dram_dt = mybir.dt.float32r
        sb_dt = mybir.dt.float32r
        mid_dt = mybir.dt.float32r
        out_dt = mybir.dt.float32

    chunks = _chunks(S)

    nc = bacc.Bacc(None)
    xt_d = nc.declare_dram_parameter("xt", [E_PER_CORE, HT, 128, S], dram_dt, isOutput=False)
    gu_d = nc.declare_dram_parameter("guw", [E_PER_CORE, HT, 128, 1024], dram_dt, isOutput=False)
    dn_d = nc.declare_dram_parameter("dnw", [E_PER_CORE, IT, 128, 1024], dram_dt, isOutput=False)
    yt_d = nc.declare_dram_parameter("yt", [E_PER_CORE, 128, HT * S], out_dt, isOutput=True)

    silu_fn = mybir.ActivationFunctionType.Silu

    # bf16 tiles are half-size; the f32r fallback needs smaller pools to fit
    # SBUF (~192 KB/partition usable)
    gu_bufs = 2 * HT if prec == "bf16" else HT + IT
    y_bufs = 4 if prec == "bf16" else 2
    with tile.TileContext(nc) as tc:
        with (
            tc.tile_pool(name="xpool", bufs=2 * HT) as xpool,
            tc.tile_pool(name="gupool", bufs=gu_bufs) as gupool,
            tc.tile_pool(name="dnpool", bufs=2 * IT) as dnpool,
            tc.tile_pool(name="midpool", bufs=2 * IT * len(chunks)) as midpool,
            # sil tiles are ACT-written; unique slots (no reuse) keep the
            # Activation instruction at a single sync-wait (AC struct limit 1)
            tc.tile_pool(name="silpool", bufs=E_PER_CORE * IT * len(chunks)) as silpool,
            tc.tile_pool(name="ypool", bufs=y_bufs) as ypool,
            tc.tile_pool(name="pgpool", bufs=3, space="PSUM") as pgpool,
            tc.tile_pool(name="pupool", bufs=3, space="PSUM") as pupool,
            tc.tile_pool(name="pypool", bufs=2, space="PSUM") as pypool,
        ):
            for e in range(E_PER_CORE):
                # interleave x / gate_up stripe loads so the first matmul can
                # start as soon as stripe 0 lands; split issue across engines
                # (DMA trigger is ~0.6us each on one sequencer)
                xts = []
                guts = []
                for h in range(HT):
                    tx = xpool.tile([128, S], sb_dt, tag="xt")
                    nc.scalar.dma_start(tx[:], xt_d[e, h])
                    xts.append(tx)
                    tg = gupool.tile([128, 1024], sb_dt, tag="gu")
                    nc.sync.dma_start(tg[:], gu_d[e, h])
                    guts.append(tg)
                dnts = []
                for i in range(IT):
                    t = dnpool.tile([128, 1024], sb_dt, tag="dn")
                    nc.sync.dma_start(t[:], dn_d[e, i])
                    dnts.append(t)

                mids = {}
                ywide = ypool.tile([128, HT * S], out_dt, tag="yo")
                for ci, (c0, cn) in enumerate(chunks):
                    for oi in range(IT):
                        pg = pgpool.tile([128, cn], FT, tag="pg")
                        pu = pupool.tile([128, cn], FT, tag="pu")
                        for h in range(HT):
                            nc.tensor.matmul(
                                pg[:],
                                guts[h][:, oi * 128 : (oi + 1) * 128],
                                xts[h][:, c0 : c0 + cn],
                                start=(h == 0),
                                stop=(h == HT - 1),
                            )
                        for h in range(HT):
                            nc.tensor.matmul(
                                pu[:],
                                guts[h][:, (IT + oi) * 128 : (IT + oi + 1) * 128],
                                xts[h][:, c0 : c0 + cn],
                                start=(h == 0),
                                stop=(h == HT - 1),
                            )
                        sil = silpool.tile([128, cn], FT, tag="sil")
                        nc.scalar.activation(sil[:], pg[:], silu_fn)
                        m = midpool.tile([128, cn], mid_dt, tag="mid")
                        nc.vector.scalar_tensor_tensor(
                            m[:], pu[:], 1.0, sil[:],
                            mybir.AluOpType.mult, mybir.AluOpType.mult,
                        )
                        mids[(ci, oi)] = m
                    for h in range(HT):
                        py = pypool.tile([128, cn], FT, tag="py")
                        for i in range(IT):
                            nc.tensor.matmul(
                                py[:],
                                dnts[i][:, h * 128 : (h + 1) * 128],
                                mids[(ci, i)][:],
                                start=(i == 0),
                                stop=(i == IT - 1),
                            )
                        nc.vector.tensor_copy(
                            ywide[:, h * S + c0 : h * S + c0 + cn], py[:]
                        )
                        if ci == len(chunks) - 1 and h % 2 == 1:
                            h0 = h - 1
                            nc.gpsimd.dma_start(
                                yt_d[e, :, h0 * S : (h + 1) * S],
                                ywide[:, h0 * S : (h + 1) * S],
                            )

    nc.finalize()
    _BUILD_CACHE[key] = nc
    return nc


def _install_trace_shims():
    """Make trace=True usable in this image: provide the NTFF hook module and
    neutralize the artifact upload (no bucket access needed for local use)."""
    import sys
    import types

    try:
        import antenv.axon_hooks  # noqa: F401
    except ImportError:
        hook = None
        try:
            from trn_agent_boot.trn_boot import _ntff_profile_via_ctypes

            hook = _ntff_profile_via_ctypes("/opt/axon/libaxon_pjrt.so")
        except Exception:
            hook = None
        mod = types.ModuleType("antenv.axon_hooks")
        mod._hook = hook
        mod.get_axon_ntff_profile_hook = lambda: mod._hook
        mod.set_axon_ntff_profile_hook = lambda h: setattr(mod, "_hook", h)
        sys.modules["antenv.axon_hooks"] = mod

    import concourse.bass_utils as bu

    orig_upload = bu.upload_artifacts

    def safe_upload(tmpdir):
        try:
            return orig_upload(tmpdir)
        except Exception:
            return tmpdir

    bu.upload_artifacts = safe_upload


def kernel(**inputs):
    from concourse.bass_utils import run_bass_kernel_spmd

    hidden = np.ascontiguousarray(np.asarray(inputs["hidden_states"], dtype=np.float32))
    idx = np.asarray(inputs["top_k_index"]).astype(np.int64)
    wts = np.asarray(inputs["top_k_weights"], dtype=np.float32)
    gup = np.asarray(inputs["gate_up_proj"], dtype=np.float32)
    dnp = np.asarray(inputs["down_proj"], dtype=np.float32)

    n_tok = hidden.shape[0]
    K = idx.shape[1]

    ve, vt, vw, vp, va, zero_w = _route(idx, wts, n_tok)
    cnts = np.bincount(ve, minlength=R)
    maxc = int(cnts.max())
    # N multiple of 64 elements keeps the PE moving-operand stream at full
    # rate (440 measured 231 ns/MM vs 448 at 202 ns/MM)
    S = max(256, ((maxc + 63) // 64) * 64)

    if PREC == "bf16":
        import ml_dtypes

        io_np = ml_dtypes.bfloat16
    else:
        io_np = np.float32

    # per-expert slices in the expert-sorted assignment arrays
    estarts = np.cumsum(cnts) - cnts

    in_maps = []
    for c in range(N_CORES):
        xt = np.zeros((E_PER_CORE, HT, 128, S), dtype=io_np)
        for le in range(E_PER_CORE):
            ge = c * E_PER_CORE + le
            s0, cnt = estarts[ge], cnts[ge]
            if cnt == 0:
                continue
            toks = vt[s0 : s0 + cnt]
            # [cnt, H] -> [H, cnt] -> tiles [HT, 128, cnt]
            xbuf = hidden[toks].T.reshape(HT, 128, cnt)
            xt[le, :, :, :cnt] = xbuf.astype(io_np)
        guw = (
            gup[c * E_PER_CORE : (c + 1) * E_PER_CORE]
            .transpose(0, 2, 1)  # [4, H, 2I]
            .reshape(E_PER_CORE, HT, 128, 1024)
            .astype(io_np)
        )
        dnw = (
            dnp[c * E_PER_CORE : (c + 1) * E_PER_CORE]
            .transpose(0, 2, 1)  # [4, I, H]
            .reshape(E_PER_CORE, IT, 128, 1024)
            .astype(io_np)
        )
        in_maps.append({"xt": np.ascontiguousarray(xt),
                        "guw": np.ascontiguousarray(guw),
                        "dnw": np.ascontiguousarray(dnw)})

    nc = _build_bass(S, PREC)

    trace = bool(int(os.environ.get("KERNEL_TRACE", "0")))
    if trace:
        _install_trace_shims()
    res = run_bass_kernel_spmd(nc, in_maps, list(range(N_CORES)), trace=trace)
    LAST_RUN["exec_time_ns"] = res.exec_time_ns
    LAST_RUN["mean_exec_time_ns"] = res.mean_exec_time_ns
    LAST_RUN["instructions_and_trace"] = res.instructions_and_trace
    LAST_RUN["profile_json"] = res.profile_json

    # ---- combine on host ----
    out = hidden * zero_w[:, None].astype(np.float32)
    acc = np.zeros((n_tok * K, H), dtype=np.float32)
    for c in range(N_CORES):
        yt = np.asarray(res.results[c]["yt"]).astype(np.float32)  # [4, 128, HT*S]
        for le in range(E_PER_CORE):
            ge = c * E_PER_CORE + le
            s0, cnt = estarts[ge], cnts[ge]
            if cnt == 0:
                continue
            # [128, HT, S] -> [HT, 128, S] -> [H, S]
            y = yt[le].reshape(128, HT, S).transpose(1, 0, 2).reshape(H, S)[:, :cnt].T
            acc[va[s0 : s0 + cnt]] = y * vw[s0 : s0 + cnt, None]
    out += acc.reshape(n_tok, K, H).sum(axis=1)
    return out



# revision 9
# speedup vs baseline: 1.7085x; 1.7085x over previous
"""Trainium2 Bass kernel for LongcatFlash MoE experts (expert-parallel, 8 cores).

Problem: T=4096 tokens, H=1024, I=512, 32 routed + 8 zero (identity) experts,
top-4 routing, per-expert capacity 768.

Strategy (sharding_hint = expert parallelism):
  - Host: compute routing (stable sort by expert, capacity clip), permute
    tokens to their expert's core (the "all-to-all"), quantize activations
    and weights to fp8 e4m3 (weights pre-scaled by 64 to clear the fp8
    subnormal cutoff), build per-core packed buffers.
  - Device (8 cores, SPMD): each core owns 4 routed experts (snake-assigned
    by token count so per-slot sizes match across cores); per expert run the
    gated MLP as DoubleRow fp8 matmuls (2 rows/cycle, 2x bf16 throughput):
        gu[o, c]  = sum_h guT[h, o] * xT[h, c]      (o = 2I rows, c = tokens)
        mid[i, c] = silu(gate[i, c]/64) * (up[i, c]/64)   -> fp8
        y[h, c]   = sum_i dnT[i, h] * mid[i, c]           (psum = 64*y)
    Tokens live on the free dim; weights are the stationary operand.
  - Host: gather per-assignment outputs, scale by router_weight/64,
    scatter-add per token, add the zero-expert weighted-identity term.
"""

import os

import ml_dtypes
import numpy as np

N_CORES = 8
R = 32  # routed experts
E_PER_CORE = R // N_CORES  # 4
CAPACITY = 768
H = 1024
I_DIM = 512
Q = H // 256  # 4 h-pairs (DoubleRow consumes 256 contraction rows per MM)
R2 = I_DIM // 256  # 2 i-pairs
SC = 64.0  # weight pre-scale (host); compensated by 1/SC on device + host

# pad per-slot token counts to a multiple of PADN (PE moving-operand rate)
PADN = int(os.environ.get("MOE_PADN", "64"))

LAST_RUN = {}  # filled with exec_time_ns etc. for test harness use

F8 = ml_dtypes.float8_e4m3


def _route(idx, wts, n_tok):
    """Replicates the reference's capacity-buffer routing exactly.

    Returns per-assignment (expert, token, weight, flat_index) for kept routed
    assignments, sorted by expert (stable), plus zero-expert weights.
    """
    K = idx.shape[1]
    A = n_tok * K
    flat_e = idx.reshape(-1).astype(np.int64)
    flat_t = np.repeat(np.arange(n_tok, dtype=np.int64), K)
    flat_w = wts.reshape(-1)
    order = np.argsort(flat_e, kind="stable")
    se = flat_e[order]
    st = flat_t[order]
    sw = flat_w[order]
    counts = np.bincount(flat_e, minlength=R + 8)
    starts = np.cumsum(counts) - counts
    pos = np.arange(A, dtype=np.int64) - starts[se]
    valid = (se < R) & (pos < CAPACITY)
    zero_w = np.where(idx >= R, wts, 0.0).sum(axis=1)
    return se[valid], st[valid], sw[valid], order[valid], zero_w


def _plan(cnts):
    """Snake-assign experts to (core, slot) by descending count; common
    per-slot sizes = max count in the slot's rank group, padded to PADN."""
    order = np.argsort(-cnts, kind="stable")
    slots = np.zeros((N_CORES, E_PER_CORE), dtype=np.int64)
    for j in range(E_PER_CORE):
        grp = order[j * N_CORES : (j + 1) * N_CORES]
        cores = range(N_CORES) if j % 2 == 0 else range(N_CORES - 1, -1, -1)
        for c, e in zip(cores, grp):
            slots[c, j] = e
    sizes = []
    for j in range(E_PER_CORE):
        m = int(cnts[slots[:, j]].max())
        m = max(PADN, ((m + PADN - 1) // PADN) * PADN)
        sizes.append(m)
    return slots, tuple(sizes)


def _chunks(S):
    """Split a slot of S tokens into PSUM-sized pieces (<=512 fp32 cols)."""
    n = (S + 511) // 512
    base = S // n
    rem = S - base * n
    out = []
    c0 = 0
    for i in range(n):
        cn = base + (1 if i < rem else 0)
        out.append((c0, cn))
        c0 += cn
    return out


_BUILD_CACHE = {}


def _build_bass(sizes):
    import concourse.bacc as bacc
    import concourse.mybir as mybir
    from concourse import tile

    if sizes in _BUILD_CACHE:
        return _BUILD_CACHE[sizes]

    FT = mybir.dt.float32
    F8D = mybir.dt.float8e4
    BF = mybir.dt.bfloat16
    DR = mybir.MatmulPerfMode.DoubleRow
    silu_fn = mybir.ActivationFunctionType.Silu
    copy_fn = mybir.ActivationFunctionType.Copy
    mult = mybir.AluOpType.mult

    NP = sum(sizes)
    offs = np.cumsum([0] + list(sizes))[:-1]
    piece_lists = [_chunks(S) for S in sizes]

    nc = bacc.Bacc(None)
    xt_d = nc.declare_dram_parameter("xt", [Q, 128, 2, NP], F8D, isOutput=False)
    gu_d = nc.declare_dram_parameter(
        "guw", [E_PER_CORE, Q, 128, 2, 1024], F8D, isOutput=False
    )
    dn_d = nc.declare_dram_parameter(
        "dnw", [E_PER_CORE, R2, 128, 2, 1024], F8D, isOutput=False
    )
    yt_d = nc.declare_dram_parameter("yt", [128, 8 * NP], BF, isOutput=True)

    n_pieces = sum(len(p) for p in piece_lists)

    with tile.TileContext(nc) as tc:
        with (
            tc.tile_pool(name="xpool", bufs=E_PER_CORE * Q) as xpool,
            tc.tile_pool(name="gupool", bufs=E_PER_CORE * Q) as gupool,
            tc.tile_pool(name="dnpool", bufs=E_PER_CORE * R2) as dnpool,
            # sil/mid tiles are uniquely slotted (no reuse): keeps ACT/DVE
            # writes at a single sync-wait and avoids WAR stalls
            tc.tile_pool(name="silpool", bufs=2 * n_pieces) as silpool,
            tc.tile_pool(name="midpool", bufs=R2 * n_pieces) as midpool,
            tc.tile_pool(name="ypool", bufs=E_PER_CORE) as ypool,
            tc.tile_pool(name="pgpool", bufs=2, space="PSUM") as pgpool,
            tc.tile_pool(name="pupool", bufs=2, space="PSUM") as pupool,
            tc.tile_pool(name="pypool", bufs=4, space="PSUM") as pypool,
        ):
            # ---- input DMA: x stripes on vector (idle at head), weights on
            # sync, interleaved by slot so slot 0 lands first ----
            xts = []
            guts = []
            dnts = []
            for j, S in enumerate(sizes):
                xq = []
                gq = []
                for q in range(Q):
                    tg = gupool.tile([128, 2, 1024], F8D, tag="gu")
                    nc.sync.dma_start(tg[:], gu_d[j, q])
                    gq.append(tg)
                    tx = xpool.tile([128, 2, S], F8D, tag="xt")
                    if j < 2:
                        eng = nc.scalar if q % 2 == 0 else nc.gpsimd
                    else:
                        eng = nc.sync
                    eng.dma_start(tx[:], xt_d[q][:, :, offs[j] : offs[j] + S])
                    xq.append(tx)
                dq = []
                for r in range(R2):
                    td = dnpool.tile([128, 2, 1024], F8D, tag="dn")
                    nc.sync.dma_start(td[:], dn_d[j, r])
                    dq.append(td)
                xts.append(xq)
                guts.append(gq)
                dnts.append(dq)

            copy_rr = [0, 1, 0, 1, 0, 1, 0, 1]  # h -> vector/scalar (PSUM readers)
            for j, S in enumerate(sizes):
                ywide = ypool.tile([128, 8 * S], BF, tag="yo")
                for c0, cn in piece_lists[j]:
                    mids = []
                    for r in range(R2):
                        mids.append(
                            midpool.tile([128, 2, cn], F8D, tag="mid", name=f"mid{j}_{c0}_{r}")
                        )
                    for oi in range(4):
                        pg = pgpool.tile([128, cn], FT, tag="pg")
                        pu = pupool.tile([128, cn], FT, tag="pu")
                        for q in range(Q):
                            nc.tensor.matmul(
                                pg[:],
                                guts[j][q][:, :, oi * 128 : (oi + 1) * 128],
                                xts[j][q][:, :, c0 : c0 + cn],
                                start=(q == 0),
                                stop=(q == Q - 1),
                                perf_mode=DR,
                            )
                        for q in range(Q):
                            nc.tensor.matmul(
                                pu[:],
                                guts[j][q][:, :, 512 + oi * 128 : 512 + (oi + 1) * 128],
                                xts[j][q][:, :, c0 : c0 + cn],
                                start=(q == 0),
                                stop=(q == Q - 1),
                                perf_mode=DR,
                            )
                        sil = silpool.tile([128, cn], FT, tag="sil")
                        nc.scalar.activation(sil[:], pg[:], silu_fn, scale=1.0 / SC)
                        nc.vector.scalar_tensor_tensor(
                            mids[oi // 2][:, oi % 2, :], pu[:], 1.0 / SC, sil[:],
                            mult, mult,
                        )
                    for h in range(8):
                        py = pypool.tile([128, cn], FT, tag="py")
                        for r in range(R2):
                            nc.tensor.matmul(
                                py[:],
                                dnts[j][r][:, :, h * 128 : (h + 1) * 128],
                                mids[r][:],
                                start=(r == 0),
                                stop=(r == R2 - 1),
                                perf_mode=DR,
                            )
                        dst = ywide[:, h * S + c0 : h * S + c0 + cn]
                        if copy_rr[h] == 0:
                            nc.vector.tensor_copy(dst, py[:])
                        else:
                            nc.scalar.activation(dst, py[:], copy_fn)
                nc.gpsimd.dma_start(
                    yt_d[:, 8 * offs[j] : 8 * offs[j] + 8 * S], ywide[:]
                )

    nc.finalize()
    _BUILD_CACHE[sizes] = nc
    return nc


def _install_trace_shims():
    """Make trace=True usable in this image: provide the NTFF hook module and
    neutralize the artifact upload (no bucket access needed for local use)."""
    import sys
    import types

    try:
        import antenv.axon_hooks  # noqa: F401
    except ImportError:
        hook = None
        try:
            from trn_agent_boot.trn_boot import _ntff_profile_via_ctypes

            hook = _ntff_profile_via_ctypes("/opt/axon/libaxon_pjrt.so")
        except Exception:
            hook = None
        mod = types.ModuleType("antenv.axon_hooks")
        mod._hook = hook
        mod.get_axon_ntff_profile_hook = lambda: mod._hook
        mod.set_axon_ntff_profile_hook = lambda h: setattr(mod, "_hook", h)
        sys.modules["antenv.axon_hooks"] = mod

    import concourse.bass_utils as bu

    orig_upload = bu.upload_artifacts

    def safe_upload(tmpdir):
        try:
            return orig_upload(tmpdir)
        except Exception:
            return tmpdir
    bu.upload_artifacts = safe_upload


def kernel(**inputs):
    from concourse.bass_utils import run_bass_kernel_spmd

    hidden = np.ascontiguousarray(np.asarray(inputs["hidden_states"], dtype=np.float32))
    idx = np.asarray(inputs["top_k_index"]).astype(np.int64)
    wts = np.asarray(inputs["top_k_weights"], dtype=np.float32)
    gup = np.asarray(inputs["gate_up_proj"], dtype=np.float32)
    dnp = np.asarray(inputs["down_proj"], dtype=np.float32)

    n_tok = hidden.shape[0]
    K = idx.shape[1]

    ve, vt, vw, va, zero_w = _route(idx, wts, n_tok)
    cnts = np.bincount(ve, minlength=R)
    estarts = np.cumsum(cnts) - cnts
    slots, sizes = _plan(cnts)
    NP = sum(sizes)
    offs = np.cumsum([0] + list(sizes))[:-1]

    # quantize once, globally
    hq = hidden.astype(F8)  # [T, H]
    guq = (gup[:R] * SC).astype(F8)  # [R, 2I, H]
    dnq = (dnp * SC).astype(F8)  # [R, H, I]

    in_maps = []
    for c in range(N_CORES):
        xt = np.zeros((Q, 128, 2, NP), dtype=F8)
        guw = np.zeros((E_PER_CORE, Q, 128, 2, 1024), dtype=F8)
        dnw = np.zeros((E_PER_CORE, R2, 128, 2, 1024), dtype=F8)
        for j in range(E_PER_CORE):
            ge = slots[c, j]
            s0, cnt = estarts[ge], cnts[ge]
            if cnt:
                toks = vt[s0 : s0 + cnt]
                # [cnt, H] -> [H, cnt] -> [Q, 2, 128, cnt] -> [Q, 128, 2, cnt]
                xb = hq[toks].T.reshape(Q, 2, 128, cnt).transpose(0, 2, 1, 3)
                xt[:, :, :, offs[j] : offs[j] + cnt] = xb
            # W [2I, H] -> W.T [H, 2I] -> [Q, 2, 128, 1024] -> [Q, 128, 2, 1024]
            guw[j] = guq[ge].T.reshape(Q, 2, 128, 1024).transpose(0, 2, 1, 3)
            # Wdn [H, I] -> Wdn.T [I, H] -> [R2, 2, 128, 1024] -> [R2, 128, 2, 1024]
            dnw[j] = dnq[ge].T.reshape(R2, 2, 128, 1024).transpose(0, 2, 1, 3)
        in_maps.append({
            "xt": np.ascontiguousarray(xt),
            "guw": np.ascontiguousarray(guw),
            "dnw": np.ascontiguousarray(dnw),
        })

    nc = _build_bass(sizes)

    trace = bool(int(os.environ.get("KERNEL_TRACE", "0")))
    if trace:
        _install_trace_shims()
    res = run_bass_kernel_spmd(nc, in_maps, list(range(N_CORES)), trace=trace)
    LAST_RUN["exec_time_ns"] = res.exec_time_ns
    LAST_RUN["mean_exec_time_ns"] = res.mean_exec_time_ns
    LAST_RUN["instructions_and_trace"] = res.instructions_and_trace
    LAST_RUN["profile_json"] = res.profile_json

    # ---- combine on host ----
    out = hidden * zero_w[:, None].astype(np.float32)
    acc = np.zeros((n_tok * K, H), dtype=np.float32)
    for c in range(N_CORES):
        yt = np.asarray(res.results[c]["yt"]).astype(np.float32)  # [128, 8*NP]
        for j in range(E_PER_CORE):
            ge = slots[c, j]
            s0, cnt = estarts[ge], cnts[ge]
            if cnt == 0:
                continue
            S = sizes[j]
            blk = yt[:, 8 * offs[j] : 8 * offs[j] + 8 * S]
            # [128, 8, S] -> [8, 128, S] -> [H, S]; psum held 64*y
            y = blk.reshape(128, 8, S).transpose(1, 0, 2).reshape(H, S)[:, :cnt].T
            acc[va[s0 : s0 + cnt]] = y * (vw[s0 : s0 + cnt, None] / SC)
    out += acc.reshape(n_tok, K, H).sum(axis=1)
    return out


# revision 10
# speedup vs baseline: 1.7319x; 1.0137x over previous
"""Trainium2 Bass kernel for LongcatFlash MoE experts (expert-parallel, 8 cores).

Problem: T=4096 tokens, H=1024, I=512, 32 routed + 8 zero (identity) experts,
top-4 routing, per-expert capacity 768.

Strategy (sharding_hint = expert parallelism):
  - Host: compute routing (stable sort by expert, capacity clip), permute
    tokens to their expert's core (the "all-to-all"), quantize activations
    and weights to fp8 e4m3 (weights pre-scaled by 64 to clear the fp8
    subnormal cutoff), build per-core packed buffers.
  - Device (8 cores, SPMD): each core owns 4 routed experts (snake-assigned
    by token count so per-slot sizes match across cores); per expert run the
    gated MLP as DoubleRow fp8 matmuls (2 rows/cycle, 2x bf16 throughput):
        gu[o, c]  = sum_h guT[h, o] * xT[h, c]      (o = 2I rows, c = tokens)
        mid[i, c] = silu(gate[i, c]/64) * (up[i, c]/64)   -> fp8
        y[h, c]   = sum_i dnT[i, h] * mid[i, c]           (psum = 64*y)
    Tokens live on the free dim; weights are the stationary operand.
  - Host: gather per-assignment outputs, scale by router_weight/64,
    scatter-add per token, add the zero-expert weighted-identity term.
"""

import os

import ml_dtypes
import numpy as np

N_CORES = 8
R = 32  # routed experts
E_PER_CORE = R // N_CORES  # 4
CAPACITY = 768
H = 1024
I_DIM = 512
Q = H // 256  # 4 h-pairs (DoubleRow consumes 256 contraction rows per MM)
R2 = I_DIM // 256  # 2 i-pairs
SC = 64.0  # weight pre-scale (host); compensated by 1/SC on device + host

# pad per-slot token counts to a multiple of PADN (PE moving-operand rate)
PADN = int(os.environ.get("MOE_PADN", "64"))

LAST_RUN = {}  # filled with exec_time_ns etc. for test harness use

F8 = ml_dtypes.float8_e4m3


def _route(idx, wts, n_tok):
    """Replicates the reference's capacity-buffer routing exactly.

    Returns per-assignment (expert, token, weight, flat_index) for kept routed
    assignments, sorted by expert (stable), plus zero-expert weights.
    """
    K = idx.shape[1]
    A = n_tok * K
    flat_e = idx.reshape(-1).astype(np.int64)
    flat_t = np.repeat(np.arange(n_tok, dtype=np.int64), K)
    flat_w = wts.reshape(-1)
    order = np.argsort(flat_e, kind="stable")
    se = flat_e[order]
    st = flat_t[order]
    sw = flat_w[order]
    counts = np.bincount(flat_e, minlength=R + 8)
    starts = np.cumsum(counts) - counts
    pos = np.arange(A, dtype=np.int64) - starts[se]
    valid = (se < R) & (pos < CAPACITY)
    zero_w = np.where(idx >= R, wts, 0.0).sum(axis=1)
    return se[valid], st[valid], sw[valid], order[valid], zero_w


def _plan(cnts):
    """Snake-assign experts to (core, slot) by descending count; common
    per-slot sizes = max count in the slot's rank group, padded to PADN."""
    order = np.argsort(-cnts, kind="stable")
    slots = np.zeros((N_CORES, E_PER_CORE), dtype=np.int64)
    for j in range(E_PER_CORE):
        grp = order[j * N_CORES : (j + 1) * N_CORES]
        cores = range(N_CORES) if j % 2 == 0 else range(N_CORES - 1, -1, -1)
        for c, e in zip(cores, grp):
            slots[c, j] = e
    sizes = []
    for j in range(E_PER_CORE):
        m = int(cnts[slots[:, j]].max())
        m = max(PADN, ((m + PADN - 1) // PADN) * PADN)
        sizes.append(m)
    return slots, tuple(sizes)


def _chunks(S):
    """Split a slot of S tokens into PSUM-sized pieces (<=512 fp32 cols)."""
    n = (S + 511) // 512
    base = S // n
    rem = S - base * n
    out = []
    c0 = 0
    for i in range(n):
        cn = base + (1 if i < rem else 0)
        out.append((c0, cn))
        c0 += cn
    return out


_BUILD_CACHE = {}


def _build_bass(sizes):
    import concourse.bacc as bacc
    import concourse.mybir as mybir
    from concourse import tile

    if sizes in _BUILD_CACHE:
        return _BUILD_CACHE[sizes]

    FT = mybir.dt.float32
    F8D = mybir.dt.float8e4
    BF = mybir.dt.bfloat16
    DR = mybir.MatmulPerfMode.DoubleRow
    silu_fn = mybir.ActivationFunctionType.Silu
    copy_fn = mybir.ActivationFunctionType.Copy
    mult = mybir.AluOpType.mult

    NP = sum(sizes)
    offs = np.cumsum([0] + list(sizes))[:-1]
    piece_lists = [_chunks(S) for S in sizes]

    nc = bacc.Bacc(None)
    xt_ds = [
        nc.declare_dram_parameter(f"xt{j}", [128, Q, 2, S], F8D, isOutput=False)
        for j, S in enumerate(sizes)
    ]
    # gu split into gate half / up half per expert (critical-path granularity)
    gu_d = nc.declare_dram_parameter(
        "guw", [E_PER_CORE, 2, 128, Q, 2, 512], F8D, isOutput=False
    )
    dn_d = nc.declare_dram_parameter(
        "dnw", [E_PER_CORE, 128, R2, 2, 1024], F8D, isOutput=False
    )
    yt_d = nc.declare_dram_parameter("yt", [128, 8 * NP], BF, isOutput=True)

    n_pieces = sum(len(p) for p in piece_lists)

    with tile.TileContext(nc) as tc:
        with (
            tc.tile_pool(name="xpool", bufs=E_PER_CORE) as xpool,
            tc.tile_pool(name="gupool", bufs=2 * E_PER_CORE) as gupool,
            tc.tile_pool(name="dnpool", bufs=E_PER_CORE) as dnpool,
            # sil/mid tiles are uniquely slotted (no reuse): keeps ACT/DVE
            # writes at a single sync-wait and avoids WAR stalls
            tc.tile_pool(name="silpool", bufs=2 * n_pieces) as silpool,
            tc.tile_pool(name="midpool", bufs=R2 * n_pieces) as midpool,
            tc.tile_pool(name="ypool", bufs=E_PER_CORE) as ypool,
            tc.tile_pool(name="pgpool", bufs=2, space="PSUM") as pgpool,
            tc.tile_pool(name="pupool", bufs=2, space="PSUM") as pupool,
            tc.tile_pool(name="pypool", bufs=4, space="PSUM") as pypool,
        ):
            # ---- input DMA: x stripes on scalar/gpsimd (idle at head),
            # weights on sync, ordered so slot 0's working set lands first ----
            xts = []
            guts = []
            dnts = []
            for j, S in enumerate(sizes):
                tx = xpool.tile([128, Q, 2, S], F8D, tag="xt", name=f"xt{j}")
                eng = nc.scalar if j == 0 else nc.gpsimd
                eng.dma_start(tx[:], xt_ds[j][:])
                xts.append(tx)
                gj = []
                for half in range(2):
                    tg = gupool.tile(
                        [128, Q, 2, 512], F8D, tag="gu", name=f"gu{j}_{half}"
                    )
                    nc.sync.dma_start(tg[:], gu_d[j, half])
                    gj.append(tg)
                guts.append(gj)
                td = dnpool.tile([128, R2, 2, 1024], F8D, tag="dn", name=f"dn{j}")
                nc.sync.dma_start(td[:], dn_d[j])
                dnts.append(td)

            copy_rr = [0, 1, 0, 1, 0, 1, 0, 1]  # h -> vector/scalar (PSUM readers)
            for j, S in enumerate(sizes):
                ywide = ypool.tile([128, 8 * S], BF, tag="yo", name=f"yw{j}")
                for c0, cn in piece_lists[j]:
                    mids = []
                    for r in range(R2):
                        mids.append(
                            midpool.tile(
                                [128, 2, cn], F8D, tag="mid", name=f"mid{j}_{c0}_{r}"
                            )
                        )
                    for oi in range(4):
                        pg = pgpool.tile([128, cn], FT, tag="pg")
                        pu = pupool.tile([128, cn], FT, tag="pu")
                        for q in range(Q):
                            nc.tensor.matmul(
                                pg[:],
                                guts[j][0][:, q, :, oi * 128 : (oi + 1) * 128],
                                xts[j][:, q, :, c0 : c0 + cn],
                                start=(q == 0),
                                stop=(q == Q - 1),
                                perf_mode=DR,
                            )
                        for q in range(Q):
                            nc.tensor.matmul(
                                pu[:],
                                guts[j][1][:, q, :, oi * 128 : (oi + 1) * 128],
                                xts[j][:, q, :, c0 : c0 + cn],
                                start=(q == 0),
                                stop=(q == Q - 1),
                                perf_mode=DR,
                            )
                        sil = silpool.tile([128, cn], FT, tag="sil")
                        nc.scalar.activation(sil[:], pg[:], silu_fn, scale=1.0 / SC)
                        nc.vector.scalar_tensor_tensor(
                            mids[oi // 2][:, oi % 2, :], pu[:], 1.0 / SC, sil[:],
                            mult, mult,
                        )
                    for h in range(8):
                        py = pypool.tile([128, cn], FT, tag="py")
                        for r in range(R2):
                            nc.tensor.matmul(
                                py[:],
                                dnts[j][:, r, :, h * 128 : (h + 1) * 128],
                                mids[r][:],
                                start=(r == 0),
                                stop=(r == R2 - 1),
                                perf_mode=DR,
                            )
                        dst = ywide[:, h * S + c0 : h * S + c0 + cn]
                        if copy_rr[h] == 0:
                            nc.vector.tensor_copy(dst, py[:])
                        else:
                            nc.scalar.activation(dst, py[:], copy_fn)
                if j < E_PER_CORE - 1:
                    nc.sync.dma_start(
                        yt_d[:, 8 * offs[j] : 8 * offs[j] + 8 * S], ywide[:]
                    )
                else:
                    # last expert: stream out per h-pair so the tail transfer
                    # overlaps the remaining copies
                    for hp in range(4):
                        lo = 8 * offs[j] + hp * 2 * S
                        nc.sync.dma_start(
                            yt_d[:, lo : lo + 2 * S],
                            ywide[:, hp * 2 * S : (hp + 1) * 2 * S],
                        )

    nc.finalize()
    _BUILD_CACHE[sizes] = nc
    return nc


def _install_trace_shims():
    """Make trace=True usable in this image: provide the NTFF hook module and
    neutralize the artifact upload (no bucket access needed for local use)."""
    import sys
    import types

    try:
        import antenv.axon_hooks  # noqa: F401
    except ImportError:
        hook = None
        try:
            from trn_agent_boot.trn_boot import _ntff_profile_via_ctypes

            hook = _ntff_profile_via_ctypes("/opt/axon/libaxon_pjrt.so")
        except Exception:
            hook = None
        mod = types.ModuleType("antenv.axon_hooks")
        mod._hook = hook
        mod.get_axon_ntff_profile_hook = lambda: mod._hook
        mod.set_axon_ntff_profile_hook = lambda h: setattr(mod, "_hook", h)
        sys.modules["antenv.axon_hooks"] = mod

    import concourse.bass_utils as bu

    orig_upload = bu.upload_artifacts

    def safe_upload(tmpdir):
        try:
            return orig_upload(tmpdir)
        except Exception:
            return tmpdir
    bu.upload_artifacts = safe_upload


def _prep_core(c, slots, sizes, cnts, estarts, vt, hq, guq, dnq):
    """Build one core's input map (fp8, DoubleRow-packed layouts)."""
    xts = {}
    guw = np.zeros((E_PER_CORE, 2, 128, Q, 2, 512), dtype=F8)
    dnw = np.zeros((E_PER_CORE, 128, R2, 2, 1024), dtype=F8)
    for j in range(E_PER_CORE):
        S = sizes[j]
        ge = slots[c, j]
        s0, cnt = estarts[ge], cnts[ge]
        xt = np.zeros((128, Q, 2, S), dtype=F8)
        if cnt:
            toks = vt[s0 : s0 + cnt]
            # [cnt, H] -> [H, cnt] -> [Q, 2, 128, cnt] -> [128, Q, 2, cnt]
            xb = hq[toks].T.reshape(Q, 2, 128, cnt).transpose(2, 0, 1, 3)
            xt[:, :, :, :cnt] = xb
        xts[f"xt{j}"] = np.ascontiguousarray(xt)
        # W [2I, H]: gate half [0:512], up half [512:1024]
        # [half, 512, H] -> [half][o, (2q+s)*128+p] -> [half, 128, Q, 2, 512]
        W = guq[ge].reshape(2, 512, Q, 2, 128)  # [half, o, q, s, p]
        guw[j] = W.transpose(0, 4, 2, 3, 1)
        # Wdn [H, I] -> [p, r, s, h]: dnw[p, r, s, h] = Wdn[h, (2r+s)*128+p]
        Wd = dnq[ge].reshape(1024, R2, 2, 128)  # [h, r, s, p]
        dnw[j] = Wd.transpose(3, 1, 2, 0)
    return {
        **xts,
        "guw": np.ascontiguousarray(guw),
        "dnw": np.ascontiguousarray(dnw),
    }


def kernel(**inputs):
    from concourse.bass_utils import run_bass_kernel_spmd

    hidden = np.ascontiguousarray(np.asarray(inputs["hidden_states"], dtype=np.float32))
    idx = np.asarray(inputs["top_k_index"]).astype(np.int64)
    wts = np.asarray(inputs["top_k_weights"], dtype=np.float32)
    gup = np.asarray(inputs["gate_up_proj"], dtype=np.float32)
    dnp = np.asarray(inputs["down_proj"], dtype=np.float32)

    n_tok = hidden.shape[0]
    K = idx.shape[1]

    ve, vt, vw, va, zero_w = _route(idx, wts, n_tok)
    cnts = np.bincount(ve, minlength=R)
    estarts = np.cumsum(cnts) - cnts
    slots, sizes = _plan(cnts)
    NP = sum(sizes)
    offs = np.cumsum([0] + list(sizes))[:-1]

    # quantize once, globally
    hq = hidden.astype(F8)  # [T, H]
    guq = (gup[:R] * SC).astype(F8)  # [R, 2I, H]
    dnq = (dnp * SC).astype(F8)  # [R, H, I]

    in_maps = [
        _prep_core(c, slots, sizes, cnts, estarts, vt, hq, guq, dnq)
        for c in range(N_CORES)
    ]

    nc = _build_bass(sizes)

    trace = bool(int(os.environ.get("KERNEL_TRACE", "0")))
    if trace:
        _install_trace_shims()
    res = run_bass_kernel_spmd(nc, in_maps, list(range(N_CORES)), trace=trace)
    LAST_RUN["exec_time_ns"] = res.exec_time_ns
    LAST_RUN["mean_exec_time_ns"] = res.mean_exec_time_ns
    LAST_RUN["instructions_and_trace"] = res.instructions_and_trace
    LAST_RUN["profile_json"] = res.profile_json

    # ---- combine on host ----
    out = hidden * zero_w[:, None].astype(np.float32)
    acc = np.zeros((n_tok * K, H), dtype=np.float32)
    for c in range(N_CORES):
        yt = np.asarray(res.results[c]["yt"]).astype(np.float32)  # [128, 8*NP]
        for j in range(E_PER_CORE):
            ge = slots[c, j]
            s0, cnt = estarts[ge], cnts[ge]
            if cnt == 0:
                continue
            S = sizes[j]
            blk = yt[:, 8 * offs[j] : 8 * offs[j] + 8 * S]
            # [128, 8, S] -> [8, 128, S] -> [H, S]; psum held 64*y
            y = blk.reshape(128, 8, S).transpose(1, 0, 2).reshape(H, S)[:, :cnt].T
            acc[va[s0 : s0 + cnt]] = y * (vw[s0 : s0 + cnt, None] / SC)
    out += acc.reshape(n_tok, K, H).sum(axis=1)
    return out


# revision 12
# speedup vs baseline: 1.7416x; 1.0056x over previous
"""Trainium2 Bass kernel for LongcatFlash MoE experts (expert-parallel, 8 cores).

Problem: T=4096 tokens, H=1024, I=512, 32 routed + 8 zero (identity) experts,
top-4 routing, per-expert capacity 768.

Strategy (sharding_hint = expert parallelism):
  - Host: compute routing (stable sort by expert, capacity clip), permute
    tokens to their expert's core (the "all-to-all"), quantize activations
    and weights to fp8 e4m3 (weights pre-scaled by 64 to clear the fp8
    subnormal cutoff), build per-core packed buffers.
  - Device (8 cores, SPMD): each core owns 4 routed experts (snake-assigned
    by token count so per-slot sizes match across cores); per expert run the
    gated MLP as DoubleRow fp8 matmuls (2 rows/cycle, 2x bf16 throughput):
        gu[o, c]  = sum_h guT[h, o] * xT[h, c]      (o = 2I rows, c = tokens)
        mid[i, c] = silu(gate[i, c]/64) * (up[i, c]/64)   -> fp8
        y[h, c]   = sum_i dnT[i, h] * mid[i, c]           (psum = 64*y)
    Tokens live on the free dim; weights are the stationary operand.
  - Host: gather per-assignment outputs, scale by router_weight/64,
    scatter-add per token, add the zero-expert weighted-identity term.
"""

import os

import ml_dtypes
import numpy as np

N_CORES = 8
R = 32  # routed experts
E_PER_CORE = R // N_CORES  # 4
CAPACITY = 768
H = 1024
I_DIM = 512
Q = H // 256  # 4 h-pairs (DoubleRow consumes 256 contraction rows per MM)
R2 = I_DIM // 256  # 2 i-pairs
SC = 64.0  # weight pre-scale (host); compensated by 1/SC on device + host

# pad per-slot token counts to a multiple of PADN (PE moving-operand rate)
PADN = int(os.environ.get("MOE_PADN", "64"))

LAST_RUN = {}  # filled with exec_time_ns etc. for test harness use

F8 = ml_dtypes.float8_e4m3


def _route(idx, wts, n_tok):
    """Replicates the reference's capacity-buffer routing exactly.

    Returns per-assignment (expert, token, weight, flat_index) for kept routed
    assignments, sorted by expert (stable), plus zero-expert weights.
    """
    K = idx.shape[1]
    A = n_tok * K
    flat_e = idx.reshape(-1).astype(np.int64)
    flat_t = np.repeat(np.arange(n_tok, dtype=np.int64), K)
    flat_w = wts.reshape(-1)
    order = np.argsort(flat_e, kind="stable")
    se = flat_e[order]
    st = flat_t[order]
    sw = flat_w[order]
    counts = np.bincount(flat_e, minlength=R + 8)
    starts = np.cumsum(counts) - counts
    pos = np.arange(A, dtype=np.int64) - starts[se]
    valid = (se < R) & (pos < CAPACITY)
    zero_w = np.where(idx >= R, wts, 0.0).sum(axis=1)
    return se[valid], st[valid], sw[valid], order[valid], zero_w


def _plan(cnts):
    """Snake-assign experts to (core, slot) by descending count; common
    per-slot sizes = max count in the slot's rank group, padded to PADN."""
    order = np.argsort(-cnts, kind="stable")
    slots = np.zeros((N_CORES, E_PER_CORE), dtype=np.int64)
    for j in range(E_PER_CORE):
        grp = order[j * N_CORES : (j + 1) * N_CORES]
        cores = range(N_CORES) if j % 2 == 0 else range(N_CORES - 1, -1, -1)
        for c, e in zip(cores, grp):
            slots[c, j] = e
    sizes = []
    for j in range(E_PER_CORE):
        m = int(cnts[slots[:, j]].max())
        m = max(PADN, ((m + PADN - 1) // PADN) * PADN)
        sizes.append(m)
    return slots, tuple(sizes)


def _chunks(S):
    """Split a slot of S tokens into PSUM-sized pieces (<=512 fp32 cols)."""
    n = (S + 511) // 512
    base = S // n
    rem = S - base * n
    out = []
    c0 = 0
    for i in range(n):
        cn = base + (1 if i < rem else 0)
        out.append((c0, cn))
        c0 += cn
    return out


_BUILD_CACHE = {}


def _build_bass(sizes):
    import concourse.bacc as bacc
    import concourse.mybir as mybir
    from concourse import tile

    if sizes in _BUILD_CACHE:
        return _BUILD_CACHE[sizes]

    FT = mybir.dt.float32
    F8D = mybir.dt.float8e4
    BF = mybir.dt.bfloat16
    DR = mybir.MatmulPerfMode.DoubleRow
    silu_fn = mybir.ActivationFunctionType.Silu
    copy_fn = mybir.ActivationFunctionType.Copy
    mult = mybir.AluOpType.mult

    NP = sum(sizes)
    offs = np.cumsum([0] + list(sizes))[:-1]
    piece_lists = [_chunks(S) for S in sizes]

    nc = bacc.Bacc(None)
    xt_ds = [
        nc.declare_dram_parameter(f"xt{j}", [128, Q, 2, S], F8D, isOutput=False)
        for j, S in enumerate(sizes)
    ]
    # gu packaged per o-tile pair [gate_oi | up_oi] so the first PSUM group
    # only needs one 256KB part
    gu_d = nc.declare_dram_parameter(
        "guw", [E_PER_CORE, 4, 128, Q, 2, 256], F8D, isOutput=False
    )
    dn_d = nc.declare_dram_parameter(
        "dnw", [E_PER_CORE, 128, R2, 2, 1024], F8D, isOutput=False
    )
    yt_d = nc.declare_dram_parameter("yt", [128, 8 * NP], BF, isOutput=True)

    n_pieces = sum(len(p) for p in piece_lists)

    with tile.TileContext(nc) as tc:
        with (
            tc.tile_pool(name="xpool", bufs=E_PER_CORE) as xpool,
            tc.tile_pool(name="gupool", bufs=4 * E_PER_CORE) as gupool,
            tc.tile_pool(name="dnpool", bufs=E_PER_CORE) as dnpool,
            # sil/mid tiles are uniquely slotted (no reuse): keeps ACT/DVE
            # writes at a single sync-wait and avoids WAR stalls
            tc.tile_pool(name="silpool", bufs=2 * n_pieces) as silpool,
            tc.tile_pool(name="midpool", bufs=R2 * n_pieces) as midpool,
            tc.tile_pool(name="ypool", bufs=E_PER_CORE) as ypool,
            tc.tile_pool(name="pgpool", bufs=2, space="PSUM") as pgpool,
            tc.tile_pool(name="pupool", bufs=2, space="PSUM") as pupool,
            tc.tile_pool(name="pypool", bufs=4, space="PSUM") as pypool,
        ):
            # DMA flood control: only slot 0/1 inputs are triggered up front.
            # Slot j>=2 weights + x queue on sync BEHIND slot (j-2)'s y-out
            # trigger, whose data wait naturally delays them; x1 is triggered
            # on scalar after the first silu. This keeps early HBM bandwidth
            # for the critical first-matmul working set (x0 + gu0 part 0).
            xts = [
                xpool.tile([128, Q, 2, S], F8D, tag="xt", name=f"xt{j}")
                for j, S in enumerate(sizes)
            ]
            guts = [
                [
                    gupool.tile([128, Q, 2, 256], F8D, tag="gu", name=f"gu{j}_{oi}")
                    for oi in range(4)
                ]
                for j in range(E_PER_CORE)
            ]
            dnts = [
                dnpool.tile([128, R2, 2, 1024], F8D, tag="dn", name=f"dn{j}")
                for j in range(E_PER_CORE)
            ]

            def load_slot_weights(j):
                for oi in range(4):
                    nc.sync.dma_start(guts[j][oi][:], gu_d[j, oi])
                nc.sync.dma_start(dnts[j][:], dn_d[j])

            nc.scalar.dma_start(xts[0][:], xt_ds[0][:])
            load_slot_weights(0)
            load_slot_weights(1)

            copy_rr = [0, 1, 0, 1, 0, 1, 0, 1]  # h -> vector/scalar (PSUM readers)
            for j, S in enumerate(sizes):
                ywide = ypool.tile([128, 8 * S], BF, tag="yo", name=f"yw{j}")
                for pi, (c0, cn) in enumerate(piece_lists[j]):
                    mids = []
                    for r in range(R2):
                        mids.append(
                            midpool.tile(
                                [128, 2, cn], F8D, tag="mid", name=f"mid{j}_{c0}_{r}"
                            )
                        )
                    for oi in range(4):
                        pg = pgpool.tile([128, cn], FT, tag="pg")
                        pu = pupool.tile([128, cn], FT, tag="pu")
                        for q in range(Q):
                            nc.tensor.matmul(
                                pg[:],
                                guts[j][oi][:, q, :, 0:128],
                                xts[j][:, q, :, c0 : c0 + cn],
                                start=(q == 0),
                                stop=(q == Q - 1),
                                perf_mode=DR,
                            )
                        for q in range(Q):
                            nc.tensor.matmul(
                                pu[:],
                                guts[j][oi][:, q, :, 128:256],
                                xts[j][:, q, :, c0 : c0 + cn],
                                start=(q == 0),
                                stop=(q == Q - 1),
                                perf_mode=DR,
                            )
                        sil = silpool.tile([128, cn], FT, tag="sil")
                        nc.scalar.activation(sil[:], pg[:], silu_fn, scale=1.0 / SC)
                        if j == 0 and pi == 0 and oi == 0:
                            nc.scalar.dma_start(xts[1][:], xt_ds[1][:])
                        nc.vector.scalar_tensor_tensor(
                            mids[oi // 2][:, oi % 2, :], pu[:], 1.0 / SC, sil[:],
                            mult, mult,
                        )
                    for h in range(8):
                        py = pypool.tile([128, cn], FT, tag="py")
                        for r in range(R2):
                            nc.tensor.matmul(
                                py[:],
                                dnts[j][:, r, :, h * 128 : (h + 1) * 128],
                                mids[r][:],
                                start=(r == 0),
                                stop=(r == R2 - 1),
                                perf_mode=DR,
                            )
                        dst = ywide[:, h * S + c0 : h * S + c0 + cn]
                        if copy_rr[h] == 0:
                            nc.vector.tensor_copy(dst, py[:])
                        else:
                            nc.scalar.activation(dst, py[:], copy_fn)
                if j < E_PER_CORE - 1:
                    # y trigger's data wait stalls sync here, which gates the
                    # slot j+2 input triggers behind slot j's completion
                    nc.sync.dma_start(
                        yt_d[:, 8 * offs[j] : 8 * offs[j] + 8 * S], ywide[:]
                    )
                    if j + 2 < E_PER_CORE:
                        nc.sync.dma_start(xts[j + 2][:], xt_ds[j + 2][:])
                        load_slot_weights(j + 2)
                else:
                    # last expert: stream out per h-pair so the tail transfer
                    # overlaps the remaining copies
                    for hp in range(4):
                        lo = 8 * offs[j] + hp * 2 * S
                        nc.sync.dma_start(
                            yt_d[:, lo : lo + 2 * S],
                            ywide[:, hp * 2 * S : (hp + 1) * 2 * S],
                        )

    nc.finalize()
    _BUILD_CACHE[sizes] = nc
    return nc


def _install_trace_shims():
    """Make trace=True usable in this image: provide the NTFF hook module and
    neutralize the artifact upload (no bucket access needed for local use)."""
    import sys
    import types

    try:
        import antenv.axon_hooks  # noqa: F401
    except ImportError:
        hook = None
        try:
            from trn_agent_boot.trn_boot import _ntff_profile_via_ctypes

            hook = _ntff_profile_via_ctypes("/opt/axon/libaxon_pjrt.so")
        except Exception:
            hook = None
        mod = types.ModuleType("antenv.axon_hooks")
        mod._hook = hook
        mod.get_axon_ntff_profile_hook = lambda: mod._hook
        mod.set_axon_ntff_profile_hook = lambda h: setattr(mod, "_hook", h)
        sys.modules["antenv.axon_hooks"] = mod

    import concourse.bass_utils as bu

    orig_upload = bu.upload_artifacts

    def safe_upload(tmpdir):
        try:
            return orig_upload(tmpdir)
        except Exception:
            return tmpdir
    bu.upload_artifacts = safe_upload


def _prep_core(c, slots, sizes, cnts, estarts, vt, hq, guq, dnq):
    """Build one core's input map (fp8, DoubleRow-packed layouts)."""
    xts = {}
    guw = np.zeros((E_PER_CORE, 4, 128, Q, 2, 256), dtype=F8)
    dnw = np.zeros((E_PER_CORE, 128, R2, 2, 1024), dtype=F8)
    for j in range(E_PER_CORE):
        S = sizes[j]
        ge = slots[c, j]
        s0, cnt = estarts[ge], cnts[ge]
        xt = np.zeros((128, Q, 2, S), dtype=F8)
        if cnt:
            toks = vt[s0 : s0 + cnt]
            # [cnt, H] -> [H, cnt] -> [Q, 2, 128, cnt] -> [128, Q, 2, cnt]
            xb = hq[toks].T.reshape(Q, 2, 128, cnt).transpose(2, 0, 1, 3)
            xt[:, :, :, :cnt] = xb
        xts[f"xt{j}"] = np.ascontiguousarray(xt)
        # W [2I, H]; part oi packs [gate rows oi*128:.. | up rows 512+oi*128:..]
        # block [o', H] -> [o', q, s, p] -> [p, q, s, o']
        for oi in range(4):
            gate = guq[ge][oi * 128 : (oi + 1) * 128]
            up = guq[ge][512 + oi * 128 : 512 + (oi + 1) * 128]
            guw[j, oi, :, :, :, 0:128] = (
                gate.reshape(128, Q, 2, 128).transpose(3, 1, 2, 0)
            )
            guw[j, oi, :, :, :, 128:256] = (
                up.reshape(128, Q, 2, 128).transpose(3, 1, 2, 0)
            )
        # Wdn [H, I] -> [p, r, s, h]: dnw[p, r, s, h] = Wdn[h, (2r+s)*128+p]
        Wd = dnq[ge].reshape(1024, R2, 2, 128)  # [h, r, s, p]
        dnw[j] = Wd.transpose(3, 1, 2, 0)
    return {
        **xts,
        "guw": np.ascontiguousarray(guw),
        "dnw": np.ascontiguousarray(dnw),
    }


def kernel(**inputs):
    from concourse.bass_utils import run_bass_kernel_spmd

    hidden = np.ascontiguousarray(np.asarray(inputs["hidden_states"], dtype=np.float32))
    idx = np.asarray(inputs["top_k_index"]).astype(np.int64)
    wts = np.asarray(inputs["top_k_weights"], dtype=np.float32)
    gup = np.asarray(inputs["gate_up_proj"], dtype=np.float32)
    dnp = np.asarray(inputs["down_proj"], dtype=np.float32)

    n_tok = hidden.shape[0]
    K = idx.shape[1]

    ve, vt, vw, va, zero_w = _route(idx, wts, n_tok)
    cnts = np.bincount(ve, minlength=R)
    estarts = np.cumsum(cnts) - cnts
    slots, sizes = _plan(cnts)
    NP = sum(sizes)
    offs = np.cumsum([0] + list(sizes))[:-1]

    # quantize once, globally
    hq = hidden.astype(F8)  # [T, H]
    guq = (gup[:R] * SC).astype(F8)  # [R, 2I, H]
    dnq = (dnp * SC).astype(F8)  # [R, H, I]

    in_maps = [
        _prep_core(c, slots, sizes, cnts, estarts, vt, hq, guq, dnq)
        for c in range(N_CORES)
    ]

    nc = _build_bass(sizes)

    trace = bool(int(os.environ.get("KERNEL_TRACE", "0")))
    if trace:
        _install_trace_shims()
    res = run_bass_kernel_spmd(nc, in_maps, list(range(N_CORES)), trace=trace)
    LAST_RUN["exec_time_ns"] = res.exec_time_ns
    LAST_RUN["mean_exec_time_ns"] = res.mean_exec_time_ns
    LAST_RUN["instructions_and_trace"] = res.instructions_and_trace
    LAST_RUN["profile_json"] = res.profile_json

    # ---- combine on host ----
    out = hidden * zero_w[:, None].astype(np.float32)
    acc = np.zeros((n_tok * K, H), dtype=np.float32)
    for c in range(N_CORES):
        yt = np.asarray(res.results[c]["yt"]).astype(np.float32)  # [128, 8*NP]
        for j in range(E_PER_CORE):
            ge = slots[c, j]
            s0, cnt = estarts[ge], cnts[ge]
            if cnt == 0:
                continue
            S = sizes[j]
            blk = yt[:, 8 * offs[j] : 8 * offs[j] + 8 * S]
            # [128, 8, S] -> [8, 128, S] -> [H, S]; psum held 64*y
            y = blk.reshape(128, 8, S).transpose(1, 0, 2).reshape(H, S)[:, :cnt].T
            acc[va[s0 : s0 + cnt]] = y * (vw[s0 : s0 + cnt, None] / SC)
    out += acc.reshape(n_tok, K, H).sum(axis=1)
    return out
